# revision 39
# baseline (speedup 1.0000x reference)
"""Differential multi-head cross-attention Trainium2 kernel (v2).

Sharding: 8 cores = 4 batches x 2 head-groups (8 heads each). Each core
computes its (batch, head-group) shard fully on device; the host sums the
two head-group partials per batch and adds the output bias.

v2 dataflow (all matmul operands bf16):
  qcT/kcT [d_cat=128, T] per head (q1/k1 rows 0-63, q2/k2 rows 64-127)
  scores psum [128 s, 2 sign, 512 t]; ONE 1024-wide exp per (sc, th)
  po FLIPPED: poT[t, 65] = e[s,t].T @ v_aug[s, 65]  (col 64 = Z)
    -> per-partition Z: reciprocal + tensor_scalar ops, no broadcasts
  w^T[t,d] = po1T*r1 - po2T*(lam*r2)  via ts + stt (accum_out = row sums)
  sumsq via tensor_tensor_reduce; partition sums via gpsimd all-reduce
  rsqrt via DVE pow(-0.5); ScalarE runs ONLY Exp/Copy (one act table)
  PE transpose w^T -> [d, t]; GN scale/bias fused into psum->sbuf copy
  y_part[t, e] = onorm.T @ out_w_slice
"""
import os
import sys

if "/opt/trn_rl_repo" not in sys.path:
    sys.path.insert(0, "/opt/trn_rl_repo")

import numpy as np

import concourse.bass as bass
import concourse.mybir as mybir
import concourse.tile as tile
from concourse import bacc
from concourse.bass import ts
from concourse.bass_utils import run_bass_kernel_spmd

F32 = mybir.dt.float32
BF16 = mybir.dt.bfloat16
FP8 = mybir.dt.float8e4
AF = mybir.ActivationFunctionType
ALU = mybir.AluOpType
DR = mybir.MatmulPerfMode.DoubleRow

SCORES_FP8 = False  # fp8e4m3 q/k + DoubleRow score matmuls (fails 2e-2 gate)
FINAL_FP8 = False   # fp8e4m3 onorm/out_w + DoubleRow final (fails 2e-2 gate)
V_FP8 = False        # fp8e4m3 encoder/Vw + DoubleRow v-proj (softmax-attenuated)

B, T, E, H, DH = 4, 1024, 1024, 16, 64
HL = 8          # local heads per core
EPS = 1e-5
NELEM = float(T * DH)   # groupnorm element count per (b, h)

MM_MODE = "bf16"
# Debug: stop after phase N (1=proj, 3=+scores/exp/po, 4=+wchain, 7=full)
PHASE = int(os.environ.get("KPHASE", "70"))

LAST_EXEC_NS = None
LAST_RESULTS = None


def build_module():
    nc = bacc.Bacc("TRN2", target_bir_lowering=False, debug=False, num_devices=8)
    mdt = BF16

    xT_d = nc.declare_dram_parameter("xT", [128, 8, T], mdt, isOutput=False)
    eT_d = nc.declare_dram_parameter("eT", [128, 8, T], mdt, isOutput=False)
    wq_d = nc.declare_dram_parameter("wq", [128, HL, 8, 128], mdt, isOutput=False)
    wk_d = nc.declare_dram_parameter("wk", [128, HL, 8, 128], mdt, isOutput=False)
    wv_d = nc.declare_dram_parameter("wv", [128, 8, 512],
                                     FP8 if V_FP8 else mdt, isOutput=False)
    if V_FP8:
        eT8_d = nc.declare_dram_parameter("eT8", [128, 8, T], FP8, isOutput=False)
    qb_d = nc.declare_dram_parameter("qb", [128, HL], F32, isOutput=False)
    kb_d = nc.declare_dram_parameter("kb", [128, HL], F32, isOutput=False)
    vb_d = nc.declare_dram_parameter("vb", [128, 512], F32, isOutput=False)
    lamb_d = nc.declare_dram_parameter("lamb", [128, HL], F32, isOutput=False)
    gamp_d = nc.declare_dram_parameter("gamp", [128, 4], F32, isOutput=False)
    betp_d = nc.declare_dram_parameter("betp", [128, 4], F32, isOutput=False)
    ow_d = nc.declare_dram_parameter("ow", [128, 4, T], mdt, isOutput=False)
    idn_d = nc.declare_dram_parameter("idn", [128, 128], mdt, isOutput=False)
    y_d = nc.declare_dram_parameter("y", [T, E], F32, isOutput=True)

    def mm(ps, lhsT, rhs, start, stop):
        nc.tensor.matmul(ps, lhsT, rhs, start=start, stop=stop)

    with tile.TileContext(nc) as tc:
        with (
            tc.tile_pool(name="const", bufs=1) as cpool,
        ):
            # ---- constants / small tensors (DMAs issued after the
            # latency-critical first input pieces, on the Act queue) ----
            qb_sb = cpool.tile([128, HL], F32)
            kb_sb = cpool.tile([128, HL], F32)
            vb_sb = cpool.tile([128, 512], F32)
            lamb_sb = cpool.tile([128, HL], F32)
            gamp_sb = cpool.tile([128, 4], F32)
            betp_sb = cpool.tile([128, 4], F32)
            idn_sb = cpool.tile([128, 128], mdt)

            # ---- persistent big tensors ----
            v_sb = cpool.tile([128, 8, HL, 65], mdt)     # [s_part, s_chunk, h, d|1]
            nc.gpsimd.memset(v_sb[:, :, :, 64:65], 1.0)
            ones128 = cpool.tile([128, 1], F32)
            nc.gpsimd.memset(ones128, 1.0)
            ones_r = cpool.tile([1, 64], F32)
            nc.gpsimd.memset(ones_r, 1.0)
            qk_dt = FP8 if SCORES_FP8 else mdt
            qc_sb = cpool.tile([128, HL, T], qk_dt)
            kc_sb = cpool.tile([128, HL, T], qk_dt)
            fin_dt = FP8 if FINAL_FP8 else mdt
            onorm = cpool.tile([128, 4, T], fin_dt)
            ow_sb = cpool.tile([128, 4, T], fin_dt)
            w_sb = cpool.tile([128, HL, 8, 64], mdt)     # [t_part, h, tc, d]
            stat_sb = cpool.tile([128, HL, 2, 8], F32)   # [t_part, h, {sum,sq}, tc]
            sar_sb = cpool.tile([128, HL, 2, 8], F32)    # all-reduced stats
            sums_sb = cpool.tile([128, HL, 2, 1], F32)
            chn_sb = cpool.tile([128, HL, 4], F32)       # mean, m2, mean^2, var
            rst_sb = cpool.tile([128, HL, 1], F32)       # rstd
            Gb_sb = cpool.tile([128, 4], F32)            # per-pair scale
            nB_sb = cpool.tile([128, 4], F32)            # per-pair mean*G - beta

            proj_pools = tc.tile_pool(name="bigin", bufs=1)
            bpool = proj_pools.__enter__()
            wpool_cm = tc.tile_pool(name="wts", bufs=1)
            wpool = wpool_cm.__enter__()
            epool_cm = tc.tile_pool(name="eps", bufs=1)
            epool = epool_cm.__enter__()
            spool_cm = tc.tile_pool(name="small", bufs=1)
            spool = spool_cm.__enter__()
            psum_cm = tc.tile_pool(name="psum", bufs=1, space="PSUM")
            psum = psum_cm.__enter__()

            xT_sb = bpool.tile([128, 8, T], mdt, tag="xT")
            eT_sb = bpool.tile([128, 8, T], mdt, tag="eT")
            wv_sb = bpool.tile([128, 8, 512], FP8 if V_FP8 else mdt, tag="wv")
            if V_FP8:
                eT8_sb = bpool.tile([128, 8, T], FP8, tag="eT8")
            # First-dependency DMAs are split small so the first proj matmuls
            # can start as soon as the leading pieces land; eT/weights stream
            # on the Pool and Activation DMA queues in parallel with xT on SP.
            wq0 = wpool.tile([128, 8, 128], mdt, tag="wq", bufs=2)
            nc.sync.dma_start(wq0[:, 0:2], wq_d[:, 0, 0:2])
            wk0 = wpool.tile([128, 8, 128], mdt, tag="wk", bufs=2)
            nc.scalar.dma_start(wk0[:, 0:2], wk_d[:, 0, 0:2])
            nc.sync.dma_start(xT_sb[:, 0, 0:512], xT_d[:, 0, 0:512])
            nc.scalar.dma_start(eT_sb[:, 0, 0:512], eT_d[:, 0, 0:512])
            nc.sync.dma_start(xT_sb[:, 0, 512:1024], xT_d[:, 0, 512:1024])
            nc.scalar.dma_start(eT_sb[:, 0, 512:1024], eT_d[:, 0, 512:1024])
            nc.sync.dma_start(wq0[:, 2:8], wq_d[:, 0, 2:8])
            nc.scalar.dma_start(wk0[:, 2:8], wk_d[:, 0, 2:8])
            nc.scalar.dma_start(qb_sb, qb_d[:])
            nc.scalar.dma_start(kb_sb, kb_d[:])
            nc.scalar.dma_start(vb_sb, vb_d[:])
            nc.scalar.dma_start(lamb_sb, lamb_d[:])
            nc.scalar.dma_start(gamp_sb, gamp_d[:])
            nc.scalar.dma_start(betp_sb, betp_d[:])
            nc.scalar.dma_start(idn_sb, idn_d[:])
            # Prime engine vector-clocks on the const DMAs so later compute
            # instructions don't each accumulate DMA-queue waits (ISA caps
            # the sync-wait count per instruction).
            prime_a = cpool.tile([128, 1], F32)
            nc.scalar.copy(prime_a, qb_sb[:, 0:1])
            nc.scalar.copy(prime_a, kb_sb[:, 0:1])
            prime_d = cpool.tile([128, 1], F32)
            nc.vector.tensor_copy(prime_d, vb_sb[:, 0:1])
            nc.vector.tensor_copy(prime_d, lamb_sb[:, 0:1])
            nc.vector.tensor_copy(prime_d, gamp_sb[:, 0:1])
            nc.vector.tensor_copy(prime_d, betp_sb[:, 0:1])
            prime_g = cpool.tile([128, 1], F32)
            nc.gpsimd.tensor_copy(prime_g, betp_sb[:, 0:1])
            for o in range(1, 8):
                nc.sync.dma_start(xT_sb[:, o], xT_d[:, o])
            for o in range(1, 8):
                nc.gpsimd.dma_start(eT_sb[:, o], eT_d[:, o])
            nc.gpsimd.dma_start(wv_sb, wv_d[:])
            if V_FP8:
                nc.gpsimd.dma_start(eT8_sb, eT8_d[:])
            nc.gpsimd.dma_start(ow_sb, ow_d[:])

            def proj_q(h, wq_t):
                for th in range(2):
                    pq = psum.tile([128, 512], F32, tag="pp", bufs=2)
                    for o in range(8):
                        mm(pq, wq_t[:, o], xT_sb[:, o, ts(th, 512)],
                           start=(o == 0), stop=(o == 7))
                    nc.vector.tensor_scalar_add(qc_sb[:, h, ts(th, 512)], pq,
                                                qb_sb[:, h:h + 1])

            def proj_k(h, wk_t):
                for th in range(2):
                    pk = psum.tile([128, 512], F32, tag="pp", bufs=2)
                    for o in range(8):
                        mm(pk, wk_t[:, o], eT_sb[:, o, ts(th, 512)],
                           start=(o == 0), stop=(o == 7))
                    nc.vector.tensor_scalar_add(kc_sb[:, h, ts(th, 512)], pk,
                                                kb_sb[:, h:h + 1])

            def proj_v(sc):
                """v projection chunk sc (natural [s, hd] layout)."""
                pv = psum.tile([128, 512], F32, tag="pp", bufs=2)
                if V_FP8:
                    # DoubleRow over E (o-pairs); wv host-scaled by 16
                    for vh in range(2):
                        for op in range(4):
                            nc.tensor.matmul(
                                pv[:, ts(vh, 256)],
                                eT8_sb[:, 2 * op:2 * op + 2, ts(sc, 128)],
                                wv_sb[:, 2 * op:2 * op + 2, ts(vh, 256)],
                                start=(op == 0), stop=(op == 3), perf_mode=DR)
                    nc.vector.scalar_tensor_tensor(
                        v_sb[:, sc, :, 0:64],
                        pv.rearrange("p (h d) -> p h d", d=64), 1.0 / 16.0,
                        vb_sb.rearrange("p (h d) -> p h d", d=64),
                        ALU.mult, ALU.add)
                else:
                    for o in range(8):
                        mm(pv, eT_sb[:, o, ts(sc, 128)], wv_sb[:, o],
                           start=(o == 0), stop=(o == 7))
                    nc.vector.tensor_tensor(
                        v_sb[:, sc, :, 0:64],
                        pv.rearrange("p (h d) -> p h d", d=64),
                        vb_sb.rearrange("p (h d) -> p h d", d=64),
                        ALU.add,
                    )

            def load_w(h):
                if h == 0:
                    return wq0, wk0
                wq_t = wpool.tile([128, 8, 128], mdt, tag="wq", bufs=2)
                nc.gpsimd.dma_start(wq_t, wq_d[:, h])
                wk_t = wpool.tile([128, 8, 128], mdt, tag="wk", bufs=2)
                nc.gpsimd.dma_start(wk_t, wk_d[:, h])
                return wq_t, wk_t

            def scores_unit(h, th, sc, e_t):
                ps = psum.tile([128, 2, 512], F32, tag="sc", bufs=2)
                if SCORES_FP8:
                    # DoubleRow with both k-tiles aliased (stride-0)
                    # computes 2*(k.T @ q); exp scale absorbs the 2x.
                    for sg in range(2):
                        pr = slice(64 * sg, 64 * sg + 64)
                        kb_ap = kc_sb[pr, h, ts(sc, 128)] \
                            .unsqueeze(1).broadcast_to([64, 2, 128])
                        for tq in range(2):
                            t0 = th * 512 + tq * 256
                            qb_ap = qc_sb[pr, h, t0:t0 + 256] \
                                .unsqueeze(1).broadcast_to([64, 2, 256])
                            nc.tensor.matmul(
                                ps[:, sg, ts(tq, 256)], kb_ap, qb_ap,
                                start=True, stop=True, perf_mode=DR)
                else:
                    mm(ps[:, 0], kc_sb[0:64, h, ts(sc, 128)],
                       qc_sb[0:64, h, ts(th, 512)], start=True, stop=True)
                    mm(ps[:, 1], kc_sb[64:128, h, ts(sc, 128)],
                       qc_sb[64:128, h, ts(th, 512)], start=True, stop=True)
                nc.scalar.activation(e_t[:, sc], ps, AF.Exp,
                                     scale=0.0625 if SCORES_FP8 else 0.125)

            def po_unit(h, tc_i, e_ts):
                th, tj = divmod(tc_i, 4)
                e_t = e_ts[th]
                po = psum.tile([128, 2, 65], F32, tag="po", bufs=2)
                for sg in range(2):
                    for sc in range(8):
                        mm(po[:, sg], e_t[:, sc, sg, ts(tj, 128)],
                           v_sb[:, sc, h], start=(sc == 0), stop=(sc == 7))
                if PHASE <= 3:
                    if h == 0:
                        pd = spool.tile([128, 130], F32, tag="pd", bufs=2)
                        nc.vector.tensor_copy(
                            pd, po.rearrange("p a b -> p (a b)"))
                        nc.sync.dma_start(y_d[ts(tc_i, 128), 0:130], pd)
                    return
                # w^T chain: r = 1/Z; w = po1*r1 - po2*(lam*r2)
                r = spool.tile([128, 2], F32, tag="r", bufs=2)
                nc.vector.reciprocal(r, po[:, :, 64])
                rl = spool.tile([128, 1], F32, tag="rl", bufs=2)
                nc.vector.tensor_tensor(rl, r[:, 1:2], lamb_sb[:, h:h + 1],
                                        ALU.mult)
                m2 = spool.tile([128, 64], F32, tag="m2", bufs=2)
                nc.vector.tensor_scalar_mul(m2, po[:, 1, 0:64], rl)
                m1 = spool.tile([128, 64], F32, tag="m1", bufs=2)
                nc.vector.tensor_scalar_mul(m1, po[:, 0, 0:64], r[:, 0:1])
                nc.vector.tensor_tensor(w_sb[:, h, tc_i], m1, m2, ALU.subtract)
                nc.vector.reduce_sum(stat_sb[:, h, 0, tc_i:tc_i + 1],
                                     w_sb[:, h, tc_i],
                                     axis=mybir.AxisListType.X)
                sqs = spool.tile([128, 64], F32, tag="sqs", bufs=2)
                nc.gpsimd.tensor_tensor(sqs, w_sb[:, h, tc_i], w_sb[:, h, tc_i],
                                        ALU.mult)
                nc.vector.reduce_sum(stat_sb[:, h, 1, tc_i:tc_i + 1], sqs,
                                     axis=mybir.AxisListType.X)

            def gn_stats(h):
                # v1-style: ones-matmul partition sum, ScalarE sqrt, ones
                # broadcast back to the pair's partition range.
                hp, hj = divmod(h, 2)
                pstat = psum.tile([1, 16], F32, tag="po", bufs=2)
                nc.tensor.matmul(pstat, ones128, stat_sb[:, h], start=True,
                                 stop=True)
                pst = spool.tile([1, 2, 8], F32, tag="pst", bufs=2)
                nc.vector.tensor_copy(pst, pstat)
                sm = spool.tile([1, 2], F32, tag="sm", bufs=2)
                nc.vector.reduce_sum(sm.unsqueeze(2), pst,
                                     axis=mybir.AxisListType.X)
                mn = spool.tile([1, 2], F32, tag="mn", bufs=2)
                nc.vector.tensor_scalar_mul(mn, sm, 1.0 / NELEM)
                var = spool.tile([1, 1], F32, tag="var", bufs=2)
                nc.vector.tensor_tensor(var, mn[0:1, 0:1], mn[0:1, 0:1],
                                        ALU.mult)
                nc.vector.tensor_tensor(var, mn[0:1, 1:2], var, ALU.subtract)
                nc.vector.tensor_scalar_add(var, var, EPS)
                std = spool.tile([1, 1], F32, tag="std", bufs=2)
                nc.scalar.activation(std, var, AF.Sqrt)
                mr = spool.tile([1, 2], F32, tag="mr", bufs=2)
                nc.vector.tensor_copy(mr[0:1, 0:1], mn[0:1, 0:1])
                nc.vector.reciprocal(mr[0:1, 1:2], std)
                # broadcast (mean, rstd) to the pair's 64-partition range
                pbc = psum.tile([64, 2], F32, tag="po", bufs=2)
                nc.tensor.matmul(pbc, ones_r, mr, start=True, stop=True)
                rr = slice(64 * hj, 64 * hj + 64)
                bc = spool.tile([128, 2], F32, tag="bc", bufs=2)
                nc.vector.tensor_copy(bc[rr], pbc)
                nc.vector.tensor_tensor(Gb_sb[rr, hp:hp + 1], bc[rr, 1:2],
                                        gamp_sb[rr, hp:hp + 1], ALU.mult)
                mg = spool.tile([128, 1], F32, tag="mg", bufs=2)
                nc.vector.tensor_tensor(mg[rr], bc[rr, 0:1],
                                        Gb_sb[rr, hp:hp + 1], ALU.mult)
                nc.vector.tensor_tensor(nB_sb[rr, hp:hp + 1], mg[rr],
                                        betp_sb[rr, hp:hp + 1], ALU.subtract)

            def tr_apply(hp):
                """transpose pair hp's w to [d,t] and apply GN scale/bias."""
                tr = psum.tile([128, 8, 128], mdt, tag="pp", bufs=2)
                for tc_i in range(8):
                    nc.tensor.transpose(tr[0:64, tc_i],
                                        w_sb[:, 2 * hp, tc_i], idn_sb)
                    nc.tensor.transpose(tr[64:128, tc_i],
                                        w_sb[:, 2 * hp + 1, tc_i], idn_sb)
                for tc_i in range(8):
                    nc.vector.tensor_scalar(
                        onorm[:, hp, ts(tc_i, 128)], tr[:, tc_i],
                        Gb_sb[:, hp:hp + 1], nB_sb[:, hp:hp + 1],
                        ALU.mult, ALU.subtract)

            # ---- slot emission: keep ScalarE's exp stream fed ----
            proj_q(0, wq0)
            proj_k(0, wk0)
            for sc in range(4):
                proj_v(sc)
            for h in range(HL):
                wq_t, wk_t = load_w(h + 1) if h + 1 < HL else (None, None)
                if PHASE < 2:
                    if wq_t is not None:
                        proj_q(h + 1, wq_t)
                        proj_k(h + 1, wk_t)
                    if h == 0:
                        for sc in range(4, 8):
                            proj_v(sc)
                    continue
                e_ts = [epool.tile([128, 8, 2, 512], mdt, tag="e", bufs=3,
                                   name=f"e_{h}_{i}")
                        for i in range(2)]
                for sc in range(8):
                    scores_unit(h, 0, sc, e_ts[0])
                if h == 0:
                    for sc in range(4, 8):
                        proj_v(sc)
                if wq_t is not None:
                    proj_q(h + 1, wq_t)
                for sc in range(8):
                    scores_unit(h, 1, sc, e_ts[1])
                if wq_t is not None:
                    proj_k(h + 1, wk_t)
                if PHASE >= 6 and h >= 2 and h % 2 == 0:
                    tr_apply(h // 2 - 1)
                if PHASE <= 2:
                    if h == 0:
                        nc.sync.dma_start(y_d[0:128, 0:512],
                                          e_ts[0][:, 0:1, :, :].bitcast(F32))
                    continue
                for tc_i in range(8):
                    po_unit(h, tc_i, e_ts)
                if PHASE >= 5:
                    gn_stats(h)
                if PHASE == 4:
                    nc.sync.dma_start(
                        y_d[ts(h, 128), 0:256],
                        w_sb[:, h].bitcast(F32).rearrange("p a b -> p (a b)"))

            if PHASE <= 1:
                for h in range(HL):
                    nc.sync.dma_start(y_d[ts(h, 128), 0:512],
                                      qc_sb[:, h, :].bitcast(F32))

            def final_tt(tt_):
                for eh in range(2):
                    py = psum.tile([128, 512], F32, tag="sc", bufs=2)
                    for o in range(4):
                        mm(py, onorm[:, o, ts(tt_, 128)],
                           ow_sb[:, o, ts(eh, 512)],
                           start=(o == 0), stop=(o == 3))
                    yt = spool.tile([128, 512], F32, tag="yt", bufs=4)
                    nc.scalar.copy(yt, py)
                    # split + alternate queues so the last transfer is short
                    nc.sync.dma_start(y_d[ts(tt_, 128), ts(2 * eh, 256)],
                                      yt[:, 0:256])
                    nc.gpsimd.dma_start(y_d[ts(tt_, 128), ts(2 * eh + 1, 256)],
                                        yt[:, 256:512])

            if PHASE >= 6:
                # pair-3 transposes, then apply+final interleaved per t-chunk
                tr3 = psum.tile([128, 8, 128], mdt, tag="pp", bufs=2)
                for tc_i in range(8):
                    nc.tensor.transpose(tr3[0:64, tc_i], w_sb[:, 6, tc_i], idn_sb)
                    nc.tensor.transpose(tr3[64:128, tc_i], w_sb[:, 7, tc_i],
                                        idn_sb)
                for tc_i in range(8):
                    nc.vector.tensor_scalar(
                        onorm[:, 3, ts(tc_i, 128)], tr3[:, tc_i],
                        Gb_sb[:, 3:4], nB_sb[:, 3:4], ALU.mult, ALU.subtract)
                    if PHASE >= 7:
                        final_tt(tc_i)
                if PHASE == 6:
                    for g in range(4):
                        nc.sync.dma_start(
                            y_d[ts(g, 256), :].rearrange(
                                "(a p) t -> p (a t)", p=128),
                            onorm[:, g].bitcast(F32))

            psum_cm.__exit__(None, None, None)
            spool_cm.__exit__(None, None, None)
            epool_cm.__exit__(None, None, None)
            wpool_cm.__exit__(None, None, None)
            proj_pools.__exit__(None, None, None)

    nc.finalize()
    return nc


_NC = None


def _get_nc():
    global _NC
    if _NC is None:
        _NC = build_module()
    return _NC


def _prep_core(c, x, eo, Wq_cat, Wk_cat, qb_cat, kb_cat, Vw, Vb, lam, gamr, betr,
               out_w, np_mdt):
    b, hg = divmod(c, 2)
    hs = slice(hg * 8, (hg + 1) * 8)

    def dev(a):
        return np.ascontiguousarray(a.astype(np_mdt))

    import ml_dtypes
    np_fp8 = ml_dtypes.float8_e4m3fn
    xT = x[b].T.reshape(8, 128, T).transpose(1, 0, 2)
    eT = eo[b].T.reshape(8, 128, T).transpose(1, 0, 2)
    wq = Wq_cat[hs].transpose(2, 0, 1).reshape(8, 128, HL, 128).transpose(1, 2, 0, 3)
    wk = Wk_cat[hs].transpose(2, 0, 1).reshape(8, 128, HL, 128).transpose(1, 2, 0, 3)
    wv = Vw[hs].reshape(512, E).T.reshape(8, 128, 512).transpose(1, 0, 2)
    ow = out_w[:, hg * 512:(hg + 1) * 512].T.reshape(4, 128, T).transpose(1, 0, 2)
    if FINAL_FP8:
        ow = (ow * 16.0).astype(np_fp8)
    # pair-packed gamma/beta: rows 0:64 = even head of pair, 64:128 = odd
    gamp = np.concatenate([gamr[hs][0::2].T, gamr[hs][1::2].T], axis=0)  # [128,4]
    betp = np.concatenate([betr[hs][0::2].T, betr[hs][1::2].T], axis=0)
    out = {
        "xT": dev(xT),
        "eT": dev(eT),
        "wq": dev(wq),
        "wk": dev(wk),
        "wv": np.ascontiguousarray((wv * 16.0).astype(np_fp8)) if V_FP8
        else dev(wv),
        "qb": np.ascontiguousarray(qb_cat[hs].T, dtype=np.float32),
        "kb": np.ascontiguousarray(kb_cat[hs].T, dtype=np.float32),
        "vb": np.ascontiguousarray(np.tile(Vb[hs].reshape(1, 512), (128, 1)),
                                   dtype=np.float32),
        "lamb": np.ascontiguousarray(np.tile(lam[hs][None, :], (128, 1)),
                                     dtype=np.float32),
        "gamp": np.ascontiguousarray(gamp, dtype=np.float32),
        "betp": np.ascontiguousarray(betp, dtype=np.float32),
        "ow": np.ascontiguousarray(ow) if FINAL_FP8 else dev(ow),
    }
    if V_FP8:
        out["eT8"] = np.ascontiguousarray(eT.astype(np_fp8))
    return out


def make_in_maps(inputs):
    x = np.asarray(inputs["x"], np.float32)
    eo = np.asarray(inputs["encoder_out"], np.float32)
    Wq_cat = np.concatenate([np.asarray(inputs["Q1w"], np.float32),
                             np.asarray(inputs["Q2w"], np.float32)], axis=1)
    Wk_cat = np.concatenate([np.asarray(inputs["K1w"], np.float32),
                             np.asarray(inputs["K2w"], np.float32)], axis=1)
    qb_cat = np.concatenate([np.asarray(inputs["Q1b"], np.float32),
                             np.asarray(inputs["Q2b"], np.float32)], axis=1)
    kb_cat = np.concatenate([np.asarray(inputs["K1b"], np.float32),
                             np.asarray(inputs["K2b"], np.float32)], axis=1)
    Vw = np.asarray(inputs["Vw"], np.float32)
    Vb = np.asarray(inputs["Vb"], np.float32)
    lam = np.asarray(inputs["lam"], np.float32)
    gamr = np.asarray(inputs["gn_gamma"], np.float32).reshape(H, DH)
    betr = np.asarray(inputs["gn_beta"], np.float32).reshape(H, DH)
    out_w = np.asarray(inputs["out_w"], np.float32)

    import ml_dtypes
    np_mdt = ml_dtypes.bfloat16

    maps = [
        _prep_core(c, x, eo, Wq_cat, Wk_cat, qb_cat, kb_cat, Vw, Vb, lam,
                   gamr, betr, out_w, np_mdt)
        for c in range(8)
    ]
    for m in maps:
        m["idn"] = np.eye(128, dtype=np_mdt)
    return maps


def kernel(**inputs):
    global LAST_EXEC_NS, LAST_RESULTS
    nc = _get_nc()
    in_maps = make_in_maps(inputs)
    res = run_bass_kernel_spmd(nc, in_maps, core_ids=list(range(8)))
    LAST_EXEC_NS = res.exec_time_ns
    LAST_RESULTS = res
    out_b = np.asarray(inputs["out_b"], np.float32)
    parts = [res.results[c]["y"] for c in range(8)]
    y = np.stack([parts[2 * b] + parts[2 * b + 1] for b in range(B)])
    y = y + out_b[None, None, :]
    return y.astype(np.float32)
